# revision 15
# baseline (speedup 1.0000x reference)
"""Trainium2 Bass kernel for a dense transformer layer (attention + FFN).

Sharding: 8 shards = (batch b, sequence half) pairs. Each core computes the
full K/V projections for its batch (2x redundant) and Q/attention/FFN for its
1024-token query slice. No cross-core communication.

On-device layout is feature-major (transposed): activations live as
[feature, token] so every matmul is lhsT.T @ rhs with natural weight layouts.

Precision: attention runs in fp8(e4m3) with DoubleRow matmuls (two 128-row
contraction chunks per instruction = 2x FLOP throughput): V/K/Q/O projections
and the ctx (E @ V) contraction. Scores stay bf16 (their contraction is only
64, so DoubleRow buys nothing) and the FFN stays bf16 (fp8 there would blow
the error budget). fp8 operands carry power-of-2 prescales folded into the
weights on the host; descales fold into the psum->SBUF copy ops.

Scheduling: the attention phase is ACT-bound (softmax exp). The projection
matmuls for pair p+1 / V are pushed through an injection queue and drip-fed
between pair p's scores/exp/ctx instructions so the in-order PE never idles
behind the exp dependency chain.
"""

import numpy as np
import ml_dtypes

B, S, D = 4, 2048, 1024
H, DH, F = 16, 64, 4096
P = 128
NCORES = 8
SQ = B * S // NCORES  # 1024 query tokens per core
DC = D // P  # 8 feature chunks
FC = F // P  # 32 ffn chunks
SKC = S // P  # 16 key chunks
NPAIR = H // 2  # 8 head pairs (2 heads per 128-feature chunk)

BF16 = ml_dtypes.bfloat16
F8 = ml_dtypes.float8_e4m3  # TRN e4m3 (max normal 240)

SW = 64.0  # fp8 weight prescale for wq/wk/wo
SV = 32.0  # fp8 prescale folded into Wv (makes ctx fp8-representable)

_CACHE = {}


def _build_program():
    import concourse.mybir as mybir
    import concourse.tile as tile
    from concourse import bacc

    f32 = mybir.dt.float32
    bf16 = mybir.dt.bfloat16
    fp8 = mybir.dt.float8e4
    AF = mybir.ActivationFunctionType
    ALU = mybir.AluOpType
    DRM = mybir.MatmulPerfMode.DoubleRow

    nc = bacc.Bacc("TRN2", target_bir_lowering=False, debug=False, num_devices=NCORES)

    xT_d = nc.dram_tensor("xT", [P, DC, S], fp8, kind="ExternalInput")
    xqT_d = nc.dram_tensor("xqT", [P, DC, SQ], fp8, kind="ExternalInput")
    xres_d = nc.dram_tensor("xres", [P, DC, SQ], f32, kind="ExternalInput")
    wq_d = nc.dram_tensor("wq", [P, DC, D], fp8, kind="ExternalInput")
    wk_d = nc.dram_tensor("wk", [P, DC, D], fp8, kind="ExternalInput")
    wv_d = nc.dram_tensor("wv", [P, DC, D], fp8, kind="ExternalInput")
    wo_d = nc.dram_tensor("wo", [P, DC, D], fp8, kind="ExternalInput")
    w1_d = nc.dram_tensor("w1", [FC, P, DC, P], bf16, kind="ExternalInput")
    w2_d = nc.dram_tensor("w2", [DC, P, FC, P], fp8, kind="ExternalInput")
    bq_d = nc.dram_tensor("bq", [P, DC], f32, kind="ExternalInput")
    bk_d = nc.dram_tensor("bk", [P, DC], f32, kind="ExternalInput")
    bvb_d = nc.dram_tensor("bvb", [P, D], bf16, kind="ExternalInput")
    b1_d = nc.dram_tensor("b1", [P, FC], f32, kind="ExternalInput")
    b2_d = nc.dram_tensor("b2", [P, DC], f32, kind="ExternalInput")
    outT_d = nc.dram_tensor("outT", [P, DC, SQ], f32, kind="ExternalOutput")

    with tile.TileContext(nc) as tc:
        with (
            tc.tile_pool(name="biasp", bufs=1) as biasp,
        ):
            bq_sb = biasp.tile([P, DC], f32)
            bk_sb = biasp.tile([P, DC], f32)
            b1_sb = biasp.tile([P, FC], f32)
            b2_sb = biasp.tile([P, DC], f32)
            nc.sync.dma_start(bq_sb[:], bq_d[:])
            nc.sync.dma_start(bk_sb[:], bk_d[:])
            nc.sync.dma_start(b1_sb[:], b1_d[:])
            nc.sync.dma_start(b2_sb[:], b2_d[:])

            with (
                tc.tile_pool(name="ctxp", bufs=1) as ctxp,
                tc.tile_pool(name="wop", bufs=1) as wop,
            ):
                # ctx carries the SV prescale (from Wv); O-proj descales
                ctxT_sb = ctxp.tile([P, DC, SQ], fp8)
                # bf16(y): input activation of the FFN
                acc_sb = ctxp.tile([P, DC, SQ], bf16)
                wo_sb = wop.tile([P, DC, D], fp8)

                # ---------------- Phase A+B: QKV projections + attention ----
                with (
                    tc.tile_pool(name="psI", bufs=1, space="PSUM") as psI,
                    tc.tile_pool(name="abp", bufs=1) as abp,
                    tc.tile_pool(name="wvp", bufs=1) as wvp,
                    tc.tile_pool(name="ws", bufs=4) as ws,
                    tc.tile_pool(name="ep", bufs=3) as ep,
                    tc.tile_pool(name="ktp", bufs=2) as ktp,
                    tc.tile_pool(name="qtp", bufs=2) as qtp,
                    tc.tile_pool(name="rp", bufs=2) as rp,
                    tc.tile_pool(name="rbp", bufs=2) as rbp,
                ):
                    xTs = [
                        abp.tile([P, DC, 512], fp8, tag=f"xT{c}", name=f"xT{c}")
                        for c in range(4)
                    ]
                    wvs = [
                        wvp.tile([P, DC, 512], fp8, tag=f"wv{c}", name=f"wv{c}")
                        for c in range(2)
                    ]
                    bvb_sb = abp.tile([P, D], bf16)
                    xqT_sb = abp.tile([P, DC, SQ], fp8)
                    # startup DMA order: first the tiles the prologue compute
                    # needs (xTs[0], wv0, wk0/wq0, xqT), then the rest.
                    for k in range(DC):
                        nc.sync.dma_start(
                            xTs[0][:, k : k + 1, :], xT_d[:, k : k + 1, 0:512]
                        )
                        nc.gpsimd.dma_start(
                            wvs[0][:, k : k + 1, :], wv_d[:, k : k + 1, 0:512]
                        )
                    wkts = {0: ws.tile([P, DC, P], fp8, tag="wchunk", name="wkt0")}
                    nc.sync.dma_start(wkts[0][:], wk_d[:, :, 0:P])
                    wqts = {0: ws.tile([P, DC, P], fp8, tag="wchunk", name="wqt0")}
                    nc.sync.dma_start(wqts[0][:], wq_d[:, :, 0:P])
                    nc.sync.dma_start(xqT_sb[:], xqT_d[:])
                    nc.sync.dma_start(bvb_sb[:], bvb_d[:])
                    for c in range(1, 4):
                        nc.sync.dma_start(xTs[c][:], xT_d[:, :, c * 512 : (c + 1) * 512])
                    nc.sync.dma_start(wvs[1][:], wv_d[:, :, 512:1024])
                    nc.sync.dma_start(wo_sb[:], wo_d[:])

                    # V, token-major: v[sk, dv] (+ ones column per head for
                    # the softmax denominator). Carries the SV prescale.
                    v_sb = abp.tile([P, SKC, H, DH + 1], fp8)
                    nc.vector.memset(v_sb[:, :, :, DH : DH + 1], 1.0)

                    # ---- injection queue: closures emitting one PE matmul
                    # (or a group's DVE tail); drip-fed into attention gaps.
                    pending = []
                    deferred_norm = []
                    done_keys = set()
                    cur_pool = [None]

                    def _run_one():
                        key, fn = pending.pop(0)
                        fn()
                        if key is not None:
                            done_keys.add(key)

                    def pump(k):
                        n = 0
                        while pending and n < k:
                            _run_one()
                            n += 1

                    def require(key):
                        while key not in done_keys and pending:
                            _run_one()

                    def drain():
                        while pending:
                            _run_one()

                    def v_group(nv, sk):
                        xt = xTs[sk // 4]
                        co = (sk % 4) * P
                        box = {}
                        items = []
                        for t in range(DC // 2):
                            def mm(t=t, box=box, xt=xt, co=co, nv=nv):
                                if t == 0:
                                    box["ps"] = cur_pool[0].tile([P, 512], f32, tag="ps", name="ps")
                                nc.tensor.matmul(
                                    box["ps"],
                                    xt[:, 2 * t : 2 * t + 2, co : co + P],
                                    wvs[nv][:, 2 * t : 2 * t + 2, :],
                                    start=(t == 0),
                                    stop=(t == DC // 2 - 1),
                                    perf_mode=DRM,
                                )
                            items.append((None, mm))

                        def tail(box=box, sk=sk, nv=nv):
                            nc.vector.tensor_add(
                                v_sb[:, sk, nv * 8 : nv * 8 + 8, 0:DH],
                                box["ps"].rearrange("p (h d) -> p h d", h=8),
                                bvb_sb[:, nv * 512 : (nv + 1) * 512].rearrange(
                                    "p (h d) -> p h d", h=8
                                ),
                            )
                        items.append((("v", nv, sk), tail))
                        return items

                    def kq_groups(p, kt, qt, wkt, wqt):
                        items = []
                        for n in range(S // 512):
                            box = {}
                            for t in range(DC // 2):
                                def mm(n=n, t=t, box=box, wkt=wkt):
                                    if t == 0:
                                        box["ps"] = cur_pool[0].tile([P, 512], f32, tag="ps", name="ps")
                                    nc.tensor.matmul(
                                        box["ps"],
                                        wkt[:, 2 * t : 2 * t + 2, :],
                                        xTs[n][:, 2 * t : 2 * t + 2, :],
                                        start=(t == 0),
                                        stop=(t == DC // 2 - 1),
                                        perf_mode=DRM,
                                    )
                                items.append((None, mm))

                            def tail(n=n, box=box, kt=kt, p=p):
                                nc.vector.tensor_scalar(
                                    kt[:, n * 512 : (n + 1) * 512],
                                    box["ps"],
                                    1.0 / SW,
                                    bk_sb[:, p : p + 1],
                                    ALU.mult,
                                    ALU.add,
                                )
                            items.append((("kn", p, n), tail))
                        for n in range(SQ // 512):
                            box = {}
                            for t in range(DC // 2):
                                def mm(n=n, t=t, box=box, wqt=wqt):
                                    if t == 0:
                                        box["ps"] = cur_pool[0].tile([P, 512], f32, tag="ps", name="ps")
                                    nc.tensor.matmul(
                                        box["ps"],
                                        wqt[:, 2 * t : 2 * t + 2, :],
                                        xqT_sb[:, 2 * t : 2 * t + 2, n * 512 : (n + 1) * 512],
                                        start=(t == 0),
                                        stop=(t == DC // 2 - 1),
                                        perf_mode=DRM,
                                    )
                                items.append((None, mm))

                            def tail(n=n, box=box, qt=qt, p=p):
                                nc.vector.tensor_scalar(
                                    qt[:, n * 512 : (n + 1) * 512],
                                    box["ps"],
                                    1.0 / SW,
                                    bq_sb[:, p : p + 1],
                                    ALU.mult,
                                    ALU.add,
                                )
                            items.append((("qn", p, n), tail))
                            if n == SQ // 512 - 1:
                                items.append((("kq", p), lambda: None))
                        return items

                    # V(nv=1) is needed from pair 4 on; queue it during pairs
                    # 1-4 (behind that pair's K/Q so kt/qt stay just-in-time).
                    V1_QUEUE = {1: range(0, 5), 2: range(5, 10), 3: range(10, 14), 4: range(14, 16)}

                    kts = {0: ktp.tile([P, S], bf16, tag="kt", name="kt")}
                    qts = {0: qtp.tile([P, SQ], bf16, tag="qt", name="qt")}

                    # Prologue: only the first K/Q chunk of pair 0 runs
                    # synchronously so exps can start ASAP; everything else
                    # (K n1-3, Q n1, all of V0) goes through the queue with
                    # fine-grained require() keys guarding each consumer.
                    cur_pool[0] = psI
                    kq0 = kq_groups(0, kts[0], qts[0], wkts[0], wqts[0])
                    for key, it in kq0[:5] + kq0[20:25]:  # K n0, Q n0
                        it()
                        if key is not None:
                            done_keys.add(key)
                    rest = kq0[5:20] + kq0[25:]
                    v0 = [v_group(0, sk) for sk in range(SKC)]
                    pending.extend(rest[0:5])        # K n1
                    pending.extend(v0[0] + v0[1])
                    pending.extend(rest[5:10])       # K n2
                    pending.extend(v0[2] + v0[3])
                    pending.extend(rest[10:15])      # K n3
                    pending.extend(v0[4] + v0[5])
                    pending.extend(rest[15:])        # Q n1 + kq key
                    for sk in range(6, SKC):
                        pending.extend(v0[sk])

                    ps_loop = tc.tile_pool(name="psS", bufs=2, space="PSUM")
                    pc_loop = tc.tile_pool(name="psC", bufs=3, space="PSUM")
                    psS = ps_loop.__enter__()
                    psC = pc_loop.__enter__()
                    for p in range(NPAIR):
                        if p + 1 < NPAIR:
                            kts[p + 1] = ktp.tile([P, S], bf16, tag="kt", name="kt")
                            qts[p + 1] = qtp.tile([P, SQ], bf16, tag="qt", name="qt")
                            wkts[p + 1] = ws.tile([P, DC, P], fp8, tag="wchunk", name="wkt")
                            nc.sync.dma_start(
                                wkts[p + 1][:], wk_d[:, :, (p + 1) * P : (p + 2) * P]
                            )
                            wqts[p + 1] = ws.tile([P, DC, P], fp8, tag="wchunk", name="wqt")
                            nc.sync.dma_start(
                                wqts[p + 1][:], wq_d[:, :, (p + 1) * P : (p + 2) * P]
                            )
                            pending.extend(
                                kq_groups(
                                    p + 1, kts[p + 1], qts[p + 1],
                                    wkts[p + 1], wqts[p + 1],
                                )
                            )
                        if p in V1_QUEUE:
                            for sk in V1_QUEUE[p]:
                                pending.extend(v_group(1, sk))
                        require(("kq", p))
                        kt = kts.pop(p)
                        qt = qts.pop(p)

                        # Attention for head pair (2p, 2p+1). Key chunks are
                        # processed two at a time (parity pairs) so the ctx
                        # contraction can run DoubleRow over 256 keys/instr.
                        for sqn in range(SQ // 512):
                            pc0 = psC.tile([P, 512], f32, tag="pc")
                            pc1 = psC.tile([P, 512], f32, tag="pc")
                            Es = {}

                            def emit_ctx(tt, Es, pc0, pc1, p):
                                require(("v", p // 4, 2 * tt + 1))
                                E = Es.pop(tt)
                                nc.tensor.matmul(
                                    pc0[:65],
                                    v_sb[:, 2 * tt : 2 * tt + 2, 2 * p, :],
                                    E[:, :, 0, :],
                                    start=(tt == 0),
                                    stop=(tt == SKC // 2 - 1),
                                    perf_mode=DRM,
                                )
                                nc.tensor.matmul(
                                    pc1[:65],
                                    v_sb[:, 2 * tt : 2 * tt + 2, 2 * p + 1, :],
                                    E[:, :, 1, :],
                                    start=(tt == 0),
                                    stop=(tt == SKC // 2 - 1),
                                    perf_mode=DRM,
                                )

                            for t in range(SKC // 2):
                                # E[key, parity, head, q] fp8 for the chunk pair
                                E = ep.tile([P, 2, 2, 512], fp8, tag="E")
                                Es[t] = E
                                sss = []
                                if p == 0:
                                    require(("kn", 0, (2 * t + 1) // 4))
                                    require(("qn", 0, sqn))
                                for par in range(2):
                                    sk = 2 * t + par
                                    ss = psS.tile([P, 2, 512], f32)
                                    sss.append(ss)
                                    nc.tensor.matmul(
                                        ss[:, 0, :],
                                        kt[0:64, sk * P : (sk + 1) * P],
                                        qt[0:64, sqn * 512 : (sqn + 1) * 512],
                                        start=True,
                                        stop=True,
                                    )
                                    nc.tensor.matmul(
                                        ss[:, 1, :],
                                        kt[64:128, sk * P : (sk + 1) * P],
                                        qt[64:128, sqn * 512 : (sqn + 1) * 512],
                                        start=True,
                                        stop=True,
                                    )
                                    if par == 0:
                                        pump(1)
                                # both exps back-to-back on ACT; ctx for the
                                # PREVIOUS chunk pair (its E is already done)
                                # keeps PE free of unsatisfied waits.
                                nc.scalar.activation(E[:, 0], sss[0], AF.Exp)
                                nc.scalar.activation(E[:, 1], sss[1], AF.Exp)
                                if t == 0:
                                    # previous sqn's tail: last ctx pair +
                                    # normalize, deferred past this sqn's
                                    # first scores/exps so ACT never stalls
                                    # at the boundary.
                                    for fn in deferred_norm:
                                        fn()
                                    deferred_norm.clear()
                                pump(1)
                                if t > 0:
                                    emit_ctx(t - 1, Es, pc0, pc1, p)
                                pump(1)

                            def normalize(p=p, sqn=sqn, pc0=pc0, pc1=pc1):
                                # softmax normalization: ctx / rowsum (approx
                                # recip is ~18 correct bits, plenty here)
                                s0 = rp.tile([1, 512], f32, tag="s", name="s0")
                                nc.vector.tensor_copy(s0, pc0[64:65, :])
                                r0 = rp.tile([1, 512], f32, tag="r", name="r0")
                                nc.vector.reciprocal_approx_fast(r0, s0)
                                rb0 = rbp.tile([64, 512], f32, tag="rb", name="rb0")
                                nc.gpsimd.partition_broadcast(rb0, r0)
                                nc.vector.tensor_mul(
                                    ctxT_sb[0:64, p, sqn * 512 : (sqn + 1) * 512],
                                    pc0[0:64, :],
                                    rb0,
                                )
                                s1 = rp.tile([1, 512], f32, tag="s", name="s1")
                                nc.vector.tensor_copy(s1, pc1[64:65, :])
                                r1 = rp.tile([1, 512], f32, tag="r", name="r1")
                                nc.vector.reciprocal_approx_fast(r1, s1)
                                rb1 = rbp.tile([64, 512], f32, tag="rb", name="rb1")
                                nc.gpsimd.partition_broadcast(rb1, r1)
                                nc.vector.tensor_mul(
                                    ctxT_sb[64:128, p, sqn * 512 : (sqn + 1) * 512],
                                    pc1[0:64, :],
                                    rb1,
                                )

                            # defer the last ctx pair + normalize into the
                            # next sqn's first slot so ACT rolls straight into
                            # the next exps at the boundary (pc ring 2 stays
                            # legal: the next sqn's first ctx emits at t=1).
                            def tail_fn(Es=Es, pc0=pc0, pc1=pc1, p=p, normalize=normalize):
                                emit_ctx(SKC // 2 - 1, Es, pc0, pc1, p)
                                normalize()

                            deferred_norm.append(tail_fn)
                    drain()
                    for fn in deferred_norm:
                        fn()
                    deferred_norm.clear()
                    pc_loop.__exit__(None, None, None)
                    ps_loop.__exit__(None, None, None)

                # ---------------- Phase C: O projection + residual ----------
                # n-outer so the first half of acc completes early and FFN1
                # can start while the second half is still being produced.
                with (
                    tc.tile_pool(name="psD", bufs=3, space="PSUM") as psD,
                    tc.tile_pool(name="ytp", bufs=1) as ytp,
                ):
                    yT_sb = ytp.tile([P, DC, SQ], f32)
                    with tc.tile_pool(name="xrp", bufs=3) as xrp:
                        for n in range(SQ // 512):
                            for m in range(DC):
                                xr = xrp.tile([P, 512], f32, tag="xr")
                                nc.sync.dma_start(
                                    xr[:], xres_d[:, m, n * 512 : (n + 1) * 512]
                                )
                                ps = psD.tile([P, 512], f32)
                                for t in range(DC // 2):
                                    nc.tensor.matmul(
                                        ps,
                                        wo_sb[:, 2 * t : 2 * t + 2, m * P : (m + 1) * P],
                                        ctxT_sb[:, 2 * t : 2 * t + 2, n * 512 : (n + 1) * 512],
                                        start=(t == 0),
                                        stop=(t == DC // 2 - 1),
                                        perf_mode=DRM,
                                    )
                                # psum carries SW*SV; descale + residual in one op
                                ysl = yT_sb[:, m, n * 512 : (n + 1) * 512]
                                nc.vector.affine_then_add(
                                    ysl, ps, xr, 1.0 / (SW * SV), 0.0
                                )
                                # bf16(y) into acc_sb: the FFN input activation
                                nc.scalar.activation(
                                    acc_sb[:, m, n * 512 : (n + 1) * 512],
                                    ysl,
                                    AF.Copy,
                                )

                    # ---------------- Phase D: FFN layer 1 + gelu -----------
                    with tc.tile_pool(name="htp", bufs=1) as htp:
                        hT_sb = htp.tile([P, FC, SQ], fp8)
                        with (
                            tc.tile_pool(name="w2s", bufs=2) as w2s,
                            tc.tile_pool(name="outp", bufs=4) as outp,
                            tc.tile_pool(name="w1s", bufs=3) as w1s,
                        ):
                            w2t0 = w2s.tile([P, FC, P], fp8, tag="w2c", name="w2t0")
                            nc.sync.dma_start(w2t0[:], w2_d[0])
                            for m in range(FC):
                                w1t = w1s.tile([P, DC, P], bf16, tag="w1c")
                                nc.sync.dma_start(w1t[:], w1_d[m])
                                for n in range(SQ // 512):
                                    ps = psD.tile([P, 512], f32)
                                    for k in range(DC):
                                        nc.tensor.matmul(
                                            ps,
                                            w1t[:, k, :],
                                            acc_sb[:, k, n * 512 : (n + 1) * 512],
                                            start=(k == 0),
                                            stop=(k == DC - 1),
                                        )
                                    nc.scalar.activation(
                                        hT_sb[:, m, n * 512 : (n + 1) * 512],
                                        ps,
                                        AF.Gelu,
                                        bias=b1_sb[:, m : m + 1],
                                    )

                            # ------------ Phase E: FFN layer 2 + residual ---
                            for m in range(DC):
                                if m == 0:
                                    w2t = w2t0
                                else:
                                    w2t = w2s.tile([P, FC, P], fp8, tag="w2c", name="w2t")
                                    nc.sync.dma_start(w2t[:], w2_d[m])
                                for n in range(SQ // 512):
                                    ps = psD.tile([P, 512], f32)
                                    for t in range(FC // 2):
                                        nc.tensor.matmul(
                                            ps,
                                            w2t[:, 2 * t : 2 * t + 2, :],
                                            hT_sb[:, 2 * t : 2 * t + 2, n * 512 : (n + 1) * 512],
                                            start=(t == 0),
                                            stop=(t == FC // 2 - 1),
                                            perf_mode=DRM,
                                        )
                                    ot = outp.tile([P, 512], f32, tag="ot")
                                    # psum carries SW; descale + bias in one op
                                    nc.vector.tensor_scalar(
                                        ot, ps, 1.0 / SW, b2_sb[:, m : m + 1],
                                        ALU.mult, ALU.add,
                                    )
                                    nc.vector.tensor_add(
                                        ot, ot, yT_sb[:, m, n * 512 : (n + 1) * 512]
                                    )
                                    nc.sync.dma_start(
                                        outT_d[:, m, n * 512 : (n + 1) * 512], ot
                                    )

    nc.compile()
    return nc


def _get_program():
    if "nc" not in _CACHE:
        _CACHE["nc"] = _build_program()
    return _CACHE["nc"]


def _wlayout(W):
    # [D_in, D_out] -> [P, D_in//P, D_out]
    return np.ascontiguousarray(
        W.reshape(W.shape[0] // P, P, W.shape[1]).transpose(1, 0, 2)
    )


def _blayout(b):
    # [D] -> [P, D//P]
    return np.ascontiguousarray(b.reshape(b.shape[0] // P, P).T)


def prepare_in_maps(x, Wq, bq, Wk, bk, Wv, bv, Wo, bo, W1, b1, W2, b2):
    x = np.asarray(x, np.float32)
    Wq = np.asarray(Wq, np.float32)
    bq = np.asarray(bq, np.float32)
    Wk = np.asarray(Wk, np.float32)
    bk = np.asarray(bk, np.float32)
    Wv = np.asarray(Wv, np.float32)
    bv = np.asarray(bv, np.float32)
    Wo = np.asarray(Wo, np.float32)
    bo = np.asarray(bo, np.float32)
    W1 = np.asarray(W1, np.float32)
    b1 = np.asarray(b1, np.float32)
    W2 = np.asarray(W2, np.float32)
    b2 = np.asarray(b2, np.float32)

    scale = DH ** -0.5
    shared = {
        "wq": _wlayout(Wq * (scale * SW)).astype(F8),
        "wk": _wlayout(Wk * SW).astype(F8),
        "wv": _wlayout(Wv * SV).astype(F8),
        "wo": _wlayout(Wo * SW).astype(F8),
        "w1": np.ascontiguousarray(
            W1.reshape(DC, P, FC, P).transpose(2, 1, 0, 3)
        ).astype(BF16),
        "w2": np.ascontiguousarray(
            (W2 * SW).reshape(FC, P, DC, P).transpose(2, 1, 0, 3)
        ).astype(F8),
        "bq": _blayout(bq * scale),
        "bk": _blayout(bk),
        "bvb": np.ascontiguousarray(np.broadcast_to(bv * SV, (P, D))).astype(BF16),
        "b1": _blayout(b1),
        "b2": _blayout(b2),
    }

    in_maps = []
    for c in range(NCORES):
        b_idx, half = divmod(c, 2)
        xb = x[b_idx]  # [S, D]
        xbT = xb.T  # [D, S]
        xT = np.ascontiguousarray(
            xbT.reshape(DC, P, S).transpose(1, 0, 2)
        ).astype(F8)
        xqT = np.ascontiguousarray(
            xbT[:, half * SQ : (half + 1) * SQ]
            .reshape(DC, P, SQ)
            .transpose(1, 0, 2)
        ).astype(F8)
        xres = np.ascontiguousarray(
            (xbT[:, half * SQ : (half + 1) * SQ] + bo[:, None])
            .reshape(DC, P, SQ)
            .transpose(1, 0, 2)
        ).astype(np.float32)
        in_maps.append(dict(shared, xT=xT, xqT=xqT, xres=xres))
    return in_maps


def assemble_out(results):
    out = np.empty((B, S, D), np.float32)
    for c in range(NCORES):
        b_idx, half = divmod(c, 2)
        outT = results[c]["outT"]  # [P, DC, SQ]
        out[b_idx, half * SQ : (half + 1) * SQ] = (
            outT.transpose(1, 0, 2).reshape(D, SQ).T
        )
    return out


def kernel(**inputs):
    from concourse.bass_utils import run_bass_kernel_spmd

    in_maps = prepare_in_maps(**inputs)
    nc = _get_program()
    res = run_bass_kernel_spmd(nc, in_maps, core_ids=list(range(NCORES)))
    return assemble_out(res.results)


# revision 16
# speedup vs baseline: 1.0101x; 1.0101x over previous
"""Trainium2 Bass kernel for a dense transformer layer (attention + FFN).

Sharding: 8 shards = (batch b, sequence half) pairs. Each core computes the
full K/V projections for its batch (2x redundant) and Q/attention/FFN for its
1024-token query slice. No cross-core communication.

On-device layout is feature-major (transposed): activations live as
[feature, token] so every matmul is lhsT.T @ rhs with natural weight layouts.

Precision: attention runs in fp8(e4m3) with DoubleRow matmuls (two 128-row
contraction chunks per instruction = 2x FLOP throughput): V/K/Q/O projections
and the ctx (E @ V) contraction. Scores stay bf16 (their contraction is only
64, so DoubleRow buys nothing) and the FFN stays bf16 (fp8 there would blow
the error budget). fp8 operands carry power-of-2 prescales folded into the
weights on the host; descales fold into the psum->SBUF copy ops.

Scheduling: the attention phase is ACT-bound (softmax exp). The projection
matmuls for pair p+1 / V are pushed through an injection queue and drip-fed
between pair p's scores/exp/ctx instructions so the in-order PE never idles
behind the exp dependency chain.
"""

import numpy as np
import ml_dtypes

B, S, D = 4, 2048, 1024
H, DH, F = 16, 64, 4096
P = 128
NCORES = 8
SQ = B * S // NCORES  # 1024 query tokens per core
DC = D // P  # 8 feature chunks
FC = F // P  # 32 ffn chunks
SKC = S // P  # 16 key chunks
NPAIR = H // 2  # 8 head pairs (2 heads per 128-feature chunk)

BF16 = ml_dtypes.bfloat16
F8 = ml_dtypes.float8_e4m3  # TRN e4m3 (max normal 240)

SW = 64.0  # fp8 weight prescale for wq/wk/wo
SV = 32.0  # fp8 prescale folded into Wv (makes ctx fp8-representable)

_CACHE = {}


def _build_program():
    import concourse.mybir as mybir
    import concourse.tile as tile
    from concourse import bacc

    f32 = mybir.dt.float32
    bf16 = mybir.dt.bfloat16
    fp8 = mybir.dt.float8e4
    AF = mybir.ActivationFunctionType
    ALU = mybir.AluOpType
    DRM = mybir.MatmulPerfMode.DoubleRow

    nc = bacc.Bacc("TRN2", target_bir_lowering=False, debug=False, num_devices=NCORES)

    xT_d = nc.dram_tensor("xT", [P, DC, S], fp8, kind="ExternalInput")
    xqT_d = nc.dram_tensor("xqT", [P, DC, SQ], fp8, kind="ExternalInput")
    xres_d = nc.dram_tensor("xres", [P, DC, SQ], f32, kind="ExternalInput")
    wq_d = nc.dram_tensor("wq", [P, DC, D], fp8, kind="ExternalInput")
    wk_d = nc.dram_tensor("wk", [P, DC, D], fp8, kind="ExternalInput")
    wv_d = nc.dram_tensor("wv", [P, DC, D], fp8, kind="ExternalInput")
    wo_d = nc.dram_tensor("wo", [P, DC, D], fp8, kind="ExternalInput")
    w1_d = nc.dram_tensor("w1", [FC, P, DC, P], bf16, kind="ExternalInput")
    w2_d = nc.dram_tensor("w2", [DC, P, FC, P], fp8, kind="ExternalInput")
    bq_d = nc.dram_tensor("bq", [P, DC], f32, kind="ExternalInput")
    bk_d = nc.dram_tensor("bk", [P, DC], f32, kind="ExternalInput")
    bvb_d = nc.dram_tensor("bvb", [P, D], bf16, kind="ExternalInput")
    b1_d = nc.dram_tensor("b1", [P, FC], f32, kind="ExternalInput")
    b2_d = nc.dram_tensor("b2", [P, DC], f32, kind="ExternalInput")
    outT_d = nc.dram_tensor("outT", [P, DC, SQ], f32, kind="ExternalOutput")

    with tile.TileContext(nc) as tc:
        with (
            tc.tile_pool(name="biasp", bufs=1) as biasp,
        ):
            bq_sb = biasp.tile([P, DC], f32)
            bk_sb = biasp.tile([P, DC], f32)
            b1_sb = biasp.tile([P, FC], f32)
            b2_sb = biasp.tile([P, DC], f32)
            nc.sync.dma_start(bq_sb[:], bq_d[:])
            nc.sync.dma_start(bk_sb[:], bk_d[:])
            nc.sync.dma_start(b1_sb[:], b1_d[:])
            nc.sync.dma_start(b2_sb[:], b2_d[:])

            with (
                tc.tile_pool(name="ctxp", bufs=1) as ctxp,
                tc.tile_pool(name="wop", bufs=1) as wop,
            ):
                # ctx carries the SV prescale (from Wv); O-proj descales
                ctxT_sb = ctxp.tile([P, DC, SQ], fp8)
                # bf16(y): input activation of the FFN
                acc_sb = ctxp.tile([P, DC, SQ], bf16)
                wo_sb = wop.tile([P, DC, D], fp8)
                yT_sb = ctxp.tile([P, DC, SQ], f32)

                # ---------------- Phase A+B: QKV projections + attention ----
                with (
                    tc.tile_pool(name="psI", bufs=1, space="PSUM") as psI,
                    tc.tile_pool(name="xrp", bufs=3) as xrp,
                    tc.tile_pool(name="abp", bufs=1) as abp,
                    tc.tile_pool(name="wvp", bufs=1) as wvp,
                    tc.tile_pool(name="ws", bufs=4) as ws,
                    tc.tile_pool(name="ep", bufs=3) as ep,
                    tc.tile_pool(name="ktp", bufs=2) as ktp,
                    tc.tile_pool(name="qtp", bufs=2) as qtp,
                    tc.tile_pool(name="rp", bufs=2) as rp,
                    tc.tile_pool(name="rbp", bufs=2) as rbp,
                ):
                    xTs = [
                        abp.tile([P, DC, 512], fp8, tag=f"xT{c}", name=f"xT{c}")
                        for c in range(4)
                    ]
                    wvs = [
                        wvp.tile([P, DC, 512], fp8, tag=f"wv{c}", name=f"wv{c}")
                        for c in range(2)
                    ]
                    bvb_sb = abp.tile([P, D], bf16)
                    xqT_sb = abp.tile([P, DC, SQ], fp8)
                    # startup DMA order: first the tiles the prologue compute
                    # needs (xTs[0], wv0, wk0/wq0, xqT), then the rest.
                    for k in range(DC):
                        nc.sync.dma_start(
                            xTs[0][:, k : k + 1, :], xT_d[:, k : k + 1, 0:512]
                        )
                        nc.gpsimd.dma_start(
                            wvs[0][:, k : k + 1, :], wv_d[:, k : k + 1, 0:512]
                        )
                    wkts = {0: ws.tile([P, DC, P], fp8, tag="wchunk", name="wkt0")}
                    nc.sync.dma_start(wkts[0][:], wk_d[:, :, 0:P])
                    wqts = {0: ws.tile([P, DC, P], fp8, tag="wchunk", name="wqt0")}
                    nc.sync.dma_start(wqts[0][:], wq_d[:, :, 0:P])
                    nc.sync.dma_start(xqT_sb[:], xqT_d[:])
                    nc.sync.dma_start(bvb_sb[:], bvb_d[:])
                    for c in range(1, 4):
                        nc.sync.dma_start(xTs[c][:], xT_d[:, :, c * 512 : (c + 1) * 512])
                    nc.sync.dma_start(wvs[1][:], wv_d[:, :, 512:1024])
                    nc.sync.dma_start(wo_sb[:], wo_d[:])

                    # V, token-major: v[sk, dv] (+ ones column per head for
                    # the softmax denominator). Carries the SV prescale.
                    v_sb = abp.tile([P, SKC, H, DH + 1], fp8)
                    nc.vector.memset(v_sb[:, :, :, DH : DH + 1], 1.0)

                    # ---- injection queue: closures emitting one PE matmul
                    # (or a group's DVE tail); drip-fed into attention gaps.
                    pending = []
                    deferred_norm = []
                    done_keys = set()
                    cur_pool = [None]

                    def _run_one():
                        key, fn = pending.pop(0)
                        fn()
                        if key is not None:
                            done_keys.add(key)

                    def pump(k):
                        n = 0
                        while pending and n < k:
                            _run_one()
                            n += 1

                    def require(key):
                        while key not in done_keys and pending:
                            _run_one()

                    def drain():
                        while pending:
                            _run_one()

                    def v_group(nv, sk):
                        xt = xTs[sk // 4]
                        co = (sk % 4) * P
                        box = {}
                        items = []
                        for t in range(DC // 2):
                            def mm(t=t, box=box, xt=xt, co=co, nv=nv):
                                if t == 0:
                                    box["ps"] = cur_pool[0].tile([P, 512], f32, tag="ps", name="ps")
                                nc.tensor.matmul(
                                    box["ps"],
                                    xt[:, 2 * t : 2 * t + 2, co : co + P],
                                    wvs[nv][:, 2 * t : 2 * t + 2, :],
                                    start=(t == 0),
                                    stop=(t == DC // 2 - 1),
                                    perf_mode=DRM,
                                )
                            items.append((None, mm))

                        def tail(box=box, sk=sk, nv=nv):
                            nc.vector.tensor_add(
                                v_sb[:, sk, nv * 8 : nv * 8 + 8, 0:DH],
                                box["ps"].rearrange("p (h d) -> p h d", h=8),
                                bvb_sb[:, nv * 512 : (nv + 1) * 512].rearrange(
                                    "p (h d) -> p h d", h=8
                                ),
                            )
                        items.append((("v", nv, sk), tail))
                        return items

                    def kq_groups(p, kt, qt, wkt, wqt):
                        items = []
                        for n in range(S // 512):
                            box = {}
                            for t in range(DC // 2):
                                def mm(n=n, t=t, box=box, wkt=wkt):
                                    if t == 0:
                                        box["ps"] = cur_pool[0].tile([P, 512], f32, tag="ps", name="ps")
                                    nc.tensor.matmul(
                                        box["ps"],
                                        wkt[:, 2 * t : 2 * t + 2, :],
                                        xTs[n][:, 2 * t : 2 * t + 2, :],
                                        start=(t == 0),
                                        stop=(t == DC // 2 - 1),
                                        perf_mode=DRM,
                                    )
                                items.append((None, mm))

                            def tail(n=n, box=box, kt=kt, p=p):
                                nc.vector.tensor_scalar(
                                    kt[:, n * 512 : (n + 1) * 512],
                                    box["ps"],
                                    1.0 / SW,
                                    bk_sb[:, p : p + 1],
                                    ALU.mult,
                                    ALU.add,
                                )
                            items.append((("kn", p, n), tail))
                        for n in range(SQ // 512):
                            box = {}
                            for t in range(DC // 2):
                                def mm(n=n, t=t, box=box, wqt=wqt):
                                    if t == 0:
                                        box["ps"] = cur_pool[0].tile([P, 512], f32, tag="ps", name="ps")
                                    nc.tensor.matmul(
                                        box["ps"],
                                        wqt[:, 2 * t : 2 * t + 2, :],
                                        xqT_sb[:, 2 * t : 2 * t + 2, n * 512 : (n + 1) * 512],
                                        start=(t == 0),
                                        stop=(t == DC // 2 - 1),
                                        perf_mode=DRM,
                                    )
                                items.append((None, mm))

                            def tail(n=n, box=box, qt=qt, p=p):
                                nc.vector.tensor_scalar(
                                    qt[:, n * 512 : (n + 1) * 512],
                                    box["ps"],
                                    1.0 / SW,
                                    bq_sb[:, p : p + 1],
                                    ALU.mult,
                                    ALU.add,
                                )
                            items.append((("qn", p, n), tail))
                            if n == SQ // 512 - 1:
                                items.append((("kq", p), lambda: None))
                        return items

                    def oa_group(m, n):
                        # partial O-projection: ctx chunks 0-5 (final once
                        # pair 5 is done); writes yT = xres + psum/(SW*SV).
                        # Phase C adds the remaining chunk pair (6,7).
                        box = {}
                        items = []
                        for t in range(3):
                            def mm(t=t, box=box, m=m, n=n):
                                if t == 0:
                                    xr = xrp.tile([P, 512], f32, tag="xr", name="xr")
                                    box["xr"] = xr
                                    nc.sync.dma_start(
                                        xr[:], xres_d[:, m, n * 512 : (n + 1) * 512]
                                    )
                                    box["ps"] = cur_pool[0].tile(
                                        [P, 512], f32, tag="ps", name="ps"
                                    )
                                nc.tensor.matmul(
                                    box["ps"],
                                    wo_sb[:, 2 * t : 2 * t + 2, m * P : (m + 1) * P],
                                    ctxT_sb[:, 2 * t : 2 * t + 2, n * 512 : (n + 1) * 512],
                                    start=(t == 0),
                                    stop=(t == 2),
                                    perf_mode=DRM,
                                )
                            items.append((None, mm))

                        def tail(box=box, m=m, n=n):
                            nc.vector.affine_then_add(
                                yT_sb[:, m, n * 512 : (n + 1) * 512],
                                box["ps"],
                                box["xr"],
                                1.0 / (SW * SV),
                                0.0,
                            )
                        items.append((("oa", m, n), tail))
                        return items

                    # V(nv=1) is needed from pair 4 on; queue it during pairs
                    # 1-4 (behind that pair's K/Q so kt/qt stay just-in-time).
                    V1_QUEUE = {1: range(0, 5), 2: range(5, 10), 3: range(10, 14), 4: range(14, 16)}

                    kts = {0: ktp.tile([P, S], bf16, tag="kt", name="kt")}
                    qts = {0: qtp.tile([P, SQ], bf16, tag="qt", name="qt")}

                    # Prologue: only the first K/Q chunk of pair 0 runs
                    # synchronously so exps can start ASAP; everything else
                    # (K n1-3, Q n1, all of V0) goes through the queue with
                    # fine-grained require() keys guarding each consumer.
                    cur_pool[0] = psI
                    kq0 = kq_groups(0, kts[0], qts[0], wkts[0], wqts[0])
                    for key, it in kq0[:5] + kq0[20:25]:  # K n0, Q n0
                        it()
                        if key is not None:
                            done_keys.add(key)
                    rest = kq0[5:20] + kq0[25:]
                    v0 = [v_group(0, sk) for sk in range(SKC)]
                    pending.extend(rest[0:5])        # K n1
                    pending.extend(v0[0] + v0[1])
                    pending.extend(rest[5:10])       # K n2
                    pending.extend(v0[2] + v0[3])
                    pending.extend(rest[10:15])      # K n3
                    pending.extend(v0[4] + v0[5])
                    pending.extend(rest[15:])        # Q n1 + kq key
                    for sk in range(6, SKC):
                        pending.extend(v0[sk])

                    ps_loop = tc.tile_pool(name="psS", bufs=2, space="PSUM")
                    pc_loop = tc.tile_pool(name="psC", bufs=3, space="PSUM")
                    psS = ps_loop.__enter__()
                    psC = pc_loop.__enter__()
                    for p in range(NPAIR):
                        if p + 1 < NPAIR:
                            kts[p + 1] = ktp.tile([P, S], bf16, tag="kt", name="kt")
                            qts[p + 1] = qtp.tile([P, SQ], bf16, tag="qt", name="qt")
                            wkts[p + 1] = ws.tile([P, DC, P], fp8, tag="wchunk", name="wkt")
                            nc.sync.dma_start(
                                wkts[p + 1][:], wk_d[:, :, (p + 1) * P : (p + 2) * P]
                            )
                            wqts[p + 1] = ws.tile([P, DC, P], fp8, tag="wchunk", name="wqt")
                            nc.sync.dma_start(
                                wqts[p + 1][:], wq_d[:, :, (p + 1) * P : (p + 2) * P]
                            )
                            pending.extend(
                                kq_groups(
                                    p + 1, kts[p + 1], qts[p + 1],
                                    wkts[p + 1], wqts[p + 1],
                                )
                            )
                        if p in V1_QUEUE:
                            for sk in V1_QUEUE[p]:
                                pending.extend(v_group(1, sk))
                        if p == NPAIR - 1:
                            for n in range(SQ // 512):
                                for m in range(DC):
                                    pending.extend(oa_group(m, n))
                        require(("kq", p))
                        kt = kts.pop(p)
                        qt = qts.pop(p)

                        # Attention for head pair (2p, 2p+1). Key chunks are
                        # processed two at a time (parity pairs) so the ctx
                        # contraction can run DoubleRow over 256 keys/instr.
                        for sqn in range(SQ // 512):
                            pc0 = psC.tile([P, 512], f32, tag="pc")
                            pc1 = psC.tile([P, 512], f32, tag="pc")
                            Es = {}

                            def emit_ctx(tt, Es, pc0, pc1, p):
                                require(("v", p // 4, 2 * tt + 1))
                                E = Es.pop(tt)
                                nc.tensor.matmul(
                                    pc0[:65],
                                    v_sb[:, 2 * tt : 2 * tt + 2, 2 * p, :],
                                    E[:, :, 0, :],
                                    start=(tt == 0),
                                    stop=(tt == SKC // 2 - 1),
                                    perf_mode=DRM,
                                )
                                nc.tensor.matmul(
                                    pc1[:65],
                                    v_sb[:, 2 * tt : 2 * tt + 2, 2 * p + 1, :],
                                    E[:, :, 1, :],
                                    start=(tt == 0),
                                    stop=(tt == SKC // 2 - 1),
                                    perf_mode=DRM,
                                )

                            for t in range(SKC // 2):
                                # E[key, parity, head, q] fp8 for the chunk pair
                                E = ep.tile([P, 2, 2, 512], fp8, tag="E")
                                Es[t] = E
                                sss = []
                                if p == 0:
                                    require(("kn", 0, (2 * t + 1) // 4))
                                    require(("qn", 0, sqn))
                                for par in range(2):
                                    sk = 2 * t + par
                                    ss = psS.tile([P, 2, 512], f32)
                                    sss.append(ss)
                                    nc.tensor.matmul(
                                        ss[:, 0, :],
                                        kt[0:64, sk * P : (sk + 1) * P],
                                        qt[0:64, sqn * 512 : (sqn + 1) * 512],
                                        start=True,
                                        stop=True,
                                    )
                                    nc.tensor.matmul(
                                        ss[:, 1, :],
                                        kt[64:128, sk * P : (sk + 1) * P],
                                        qt[64:128, sqn * 512 : (sqn + 1) * 512],
                                        start=True,
                                        stop=True,
                                    )
                                    if par == 0:
                                        pump(1)
                                # both exps back-to-back on ACT; ctx for the
                                # PREVIOUS chunk pair (its E is already done)
                                # keeps PE free of unsatisfied waits.
                                nc.scalar.activation(E[:, 0], sss[0], AF.Exp)
                                nc.scalar.activation(E[:, 1], sss[1], AF.Exp)
                                if t == 0:
                                    # previous sqn's tail: last ctx pair +
                                    # normalize, deferred past this sqn's
                                    # first scores/exps so ACT never stalls
                                    # at the boundary.
                                    for fn in deferred_norm:
                                        fn()
                                    deferred_norm.clear()
                                pump(1)
                                if t > 0:
                                    emit_ctx(t - 1, Es, pc0, pc1, p)
                                pump(1)

                            def normalize(p=p, sqn=sqn, pc0=pc0, pc1=pc1):
                                # softmax normalization: ctx / rowsum (approx
                                # recip is ~18 correct bits, plenty here)
                                s0 = rp.tile([1, 512], f32, tag="s", name="s0")
                                nc.vector.tensor_copy(s0, pc0[64:65, :])
                                r0 = rp.tile([1, 512], f32, tag="r", name="r0")
                                nc.vector.reciprocal_approx_fast(r0, s0)
                                rb0 = rbp.tile([64, 512], f32, tag="rb", name="rb0")
                                nc.gpsimd.partition_broadcast(rb0, r0)
                                nc.vector.tensor_mul(
                                    ctxT_sb[0:64, p, sqn * 512 : (sqn + 1) * 512],
                                    pc0[0:64, :],
                                    rb0,
                                )
                                s1 = rp.tile([1, 512], f32, tag="s", name="s1")
                                nc.vector.tensor_copy(s1, pc1[64:65, :])
                                r1 = rp.tile([1, 512], f32, tag="r", name="r1")
                                nc.vector.reciprocal_approx_fast(r1, s1)
                                rb1 = rbp.tile([64, 512], f32, tag="rb", name="rb1")
                                nc.gpsimd.partition_broadcast(rb1, r1)
                                nc.vector.tensor_mul(
                                    ctxT_sb[64:128, p, sqn * 512 : (sqn + 1) * 512],
                                    pc1[0:64, :],
                                    rb1,
                                )

                            # defer the last ctx pair + normalize into the
                            # next sqn's first slot so ACT rolls straight into
                            # the next exps at the boundary (pc ring 2 stays
                            # legal: the next sqn's first ctx emits at t=1).
                            def tail_fn(Es=Es, pc0=pc0, pc1=pc1, p=p, normalize=normalize):
                                emit_ctx(SKC // 2 - 1, Es, pc0, pc1, p)
                                normalize()

                            deferred_norm.append(tail_fn)
                    drain()
                    for fn in deferred_norm:
                        fn()
                    deferred_norm.clear()
                    pc_loop.__exit__(None, None, None)
                    ps_loop.__exit__(None, None, None)

                # ---------------- Phase C: O projection + residual ----------
                # n-outer so the first half of acc completes early and FFN1
                # can start while the second half is still being produced.
                with (
                    tc.tile_pool(name="psD", bufs=3, space="PSUM") as psD,
                ):
                    for n in range(SQ // 512):
                        for m in range(DC):
                            ps = psD.tile([P, 512], f32)
                            t = DC // 2 - 1
                            nc.tensor.matmul(
                                ps,
                                wo_sb[:, 2 * t : 2 * t + 2, m * P : (m + 1) * P],
                                ctxT_sb[:, 2 * t : 2 * t + 2, n * 512 : (n + 1) * 512],
                                start=True,
                                stop=True,
                                perf_mode=DRM,
                            )
                            # add the last chunk pair onto the partial y
                            ysl = yT_sb[:, m, n * 512 : (n + 1) * 512]
                            nc.vector.affine_then_add(
                                ysl, ps, ysl, 1.0 / (SW * SV), 0.0
                            )
                            # bf16(y) into acc_sb: the FFN input activation
                            nc.scalar.activation(
                                acc_sb[:, m, n * 512 : (n + 1) * 512],
                                ysl,
                                AF.Copy,
                            )

                    # ---------------- Phase D: FFN layer 1 + gelu -----------
                    with tc.tile_pool(name="htp", bufs=1) as htp:
                        hT_sb = htp.tile([P, FC, SQ], fp8)
                        with (
                            tc.tile_pool(name="w2s", bufs=2) as w2s,
                            tc.tile_pool(name="outp", bufs=4) as outp,
                            tc.tile_pool(name="w1s", bufs=3) as w1s,
                        ):
                            w2t0 = w2s.tile([P, FC, P], fp8, tag="w2c", name="w2t0")
                            nc.sync.dma_start(w2t0[:], w2_d[0])
                            for m in range(FC):
                                w1t = w1s.tile([P, DC, P], bf16, tag="w1c")
                                nc.sync.dma_start(w1t[:], w1_d[m])
                                for n in range(SQ // 512):
                                    ps = psD.tile([P, 512], f32)
                                    for k in range(DC):
                                        nc.tensor.matmul(
                                            ps,
                                            w1t[:, k, :],
                                            acc_sb[:, k, n * 512 : (n + 1) * 512],
                                            start=(k == 0),
                                            stop=(k == DC - 1),
                                        )
                                    nc.scalar.activation(
                                        hT_sb[:, m, n * 512 : (n + 1) * 512],
                                        ps,
                                        AF.Gelu,
                                        bias=b1_sb[:, m : m + 1],
                                    )

                            # ------------ Phase E: FFN layer 2 + residual ---
                            for m in range(DC):
                                if m == 0:
                                    w2t = w2t0
                                else:
                                    w2t = w2s.tile([P, FC, P], fp8, tag="w2c", name="w2t")
                                    nc.sync.dma_start(w2t[:], w2_d[m])
                                for n in range(SQ // 512):
                                    ps = psD.tile([P, 512], f32)
                                    for t in range(FC // 2):
                                        nc.tensor.matmul(
                                            ps,
                                            w2t[:, 2 * t : 2 * t + 2, :],
                                            hT_sb[:, 2 * t : 2 * t + 2, n * 512 : (n + 1) * 512],
                                            start=(t == 0),
                                            stop=(t == FC // 2 - 1),
                                            perf_mode=DRM,
                                        )
                                    ot = outp.tile([P, 512], f32, tag="ot")
                                    # psum carries SW; descale + bias in one op
                                    nc.vector.tensor_scalar(
                                        ot, ps, 1.0 / SW, b2_sb[:, m : m + 1],
                                        ALU.mult, ALU.add,
                                    )
                                    nc.vector.tensor_add(
                                        ot, ot, yT_sb[:, m, n * 512 : (n + 1) * 512]
                                    )
                                    nc.sync.dma_start(
                                        outT_d[:, m, n * 512 : (n + 1) * 512], ot
                                    )

    nc.compile()
    return nc


def _get_program():
    if "nc" not in _CACHE:
        _CACHE["nc"] = _build_program()
    return _CACHE["nc"]


def _wlayout(W):
    # [D_in, D_out] -> [P, D_in//P, D_out]
    return np.ascontiguousarray(
        W.reshape(W.shape[0] // P, P, W.shape[1]).transpose(1, 0, 2)
    )


def _blayout(b):
    # [D] -> [P, D//P]
    return np.ascontiguousarray(b.reshape(b.shape[0] // P, P).T)


def prepare_in_maps(x, Wq, bq, Wk, bk, Wv, bv, Wo, bo, W1, b1, W2, b2):
    x = np.asarray(x, np.float32)
    Wq = np.asarray(Wq, np.float32)
    bq = np.asarray(bq, np.float32)
    Wk = np.asarray(Wk, np.float32)
    bk = np.asarray(bk, np.float32)
    Wv = np.asarray(Wv, np.float32)
    bv = np.asarray(bv, np.float32)
    Wo = np.asarray(Wo, np.float32)
    bo = np.asarray(bo, np.float32)
    W1 = np.asarray(W1, np.float32)
    b1 = np.asarray(b1, np.float32)
    W2 = np.asarray(W2, np.float32)
    b2 = np.asarray(b2, np.float32)

    scale = DH ** -0.5
    shared = {
        "wq": _wlayout(Wq * (scale * SW)).astype(F8),
        "wk": _wlayout(Wk * SW).astype(F8),
        "wv": _wlayout(Wv * SV).astype(F8),
        "wo": _wlayout(Wo * SW).astype(F8),
        "w1": np.ascontiguousarray(
            W1.reshape(DC, P, FC, P).transpose(2, 1, 0, 3)
        ).astype(BF16),
        "w2": np.ascontiguousarray(
            (W2 * SW).reshape(FC, P, DC, P).transpose(2, 1, 0, 3)
        ).astype(F8),
        "bq": _blayout(bq * scale),
        "bk": _blayout(bk),
        "bvb": np.ascontiguousarray(np.broadcast_to(bv * SV, (P, D))).astype(BF16),
        "b1": _blayout(b1),
        "b2": _blayout(b2),
    }

    in_maps = []
    for c in range(NCORES):
        b_idx, half = divmod(c, 2)
        xb = x[b_idx]  # [S, D]
        xbT = xb.T  # [D, S]
        xT = np.ascontiguousarray(
            xbT.reshape(DC, P, S).transpose(1, 0, 2)
        ).astype(F8)
        xqT = np.ascontiguousarray(
            xbT[:, half * SQ : (half + 1) * SQ]
            .reshape(DC, P, SQ)
            .transpose(1, 0, 2)
        ).astype(F8)
        xres = np.ascontiguousarray(
            (xbT[:, half * SQ : (half + 1) * SQ] + bo[:, None])
            .reshape(DC, P, SQ)
            .transpose(1, 0, 2)
        ).astype(np.float32)
        in_maps.append(dict(shared, xT=xT, xqT=xqT, xres=xres))
    return in_maps


def assemble_out(results):
    out = np.empty((B, S, D), np.float32)
    for c in range(NCORES):
        b_idx, half = divmod(c, 2)
        outT = results[c]["outT"]  # [P, DC, SQ]
        out[b_idx, half * SQ : (half + 1) * SQ] = (
            outT.transpose(1, 0, 2).reshape(D, SQ).T
        )
    return out


def kernel(**inputs):
    from concourse.bass_utils import run_bass_kernel_spmd

    in_maps = prepare_in_maps(**inputs)
    nc = _get_program()
    res = run_bass_kernel_spmd(nc, in_maps, core_ids=list(range(NCORES)))
    return assemble_out(res.results)


# revision 17
# speedup vs baseline: 1.0185x; 1.0083x over previous
"""Trainium2 Bass kernel for a dense transformer layer (attention + FFN).

Sharding: 8 shards = (batch b, sequence half) pairs. Each core computes the
full K/V projections for its batch (2x redundant) and Q/attention/FFN for its
1024-token query slice. No cross-core communication.

On-device layout is feature-major (transposed): activations live as
[feature, token] so every matmul is lhsT.T @ rhs with natural weight layouts.

Precision: attention runs in fp8(e4m3) with DoubleRow matmuls (two 128-row
contraction chunks per instruction = 2x FLOP throughput): V/K/Q/O projections
and the ctx (E @ V) contraction. Scores stay bf16 (their contraction is only
64, so DoubleRow buys nothing) and the FFN stays bf16 (fp8 there would blow
the error budget). fp8 operands carry power-of-2 prescales folded into the
weights on the host; descales fold into the psum->SBUF copy ops.

Scheduling: the attention phase is ACT-bound (softmax exp). The projection
matmuls for pair p+1 / V are pushed through an injection queue and drip-fed
between pair p's scores/exp/ctx instructions so the in-order PE never idles
behind the exp dependency chain.
"""

import numpy as np
import ml_dtypes

B, S, D = 4, 2048, 1024
H, DH, F = 16, 64, 4096
P = 128
NCORES = 8
SQ = B * S // NCORES  # 1024 query tokens per core
DC = D // P  # 8 feature chunks
FC = F // P  # 32 ffn chunks
SKC = S // P  # 16 key chunks
NPAIR = H // 2  # 8 head pairs (2 heads per 128-feature chunk)

BF16 = ml_dtypes.bfloat16
F8 = ml_dtypes.float8_e4m3  # TRN e4m3 (max normal 240)

SW = 64.0  # fp8 weight prescale for wq/wk/wo
SV = 32.0  # fp8 prescale folded into Wv (makes ctx fp8-representable)

_CACHE = {}


def _build_program():
    import concourse.mybir as mybir
    import concourse.tile as tile
    from concourse import bacc

    f32 = mybir.dt.float32
    bf16 = mybir.dt.bfloat16
    fp8 = mybir.dt.float8e4
    AF = mybir.ActivationFunctionType
    ALU = mybir.AluOpType
    DRM = mybir.MatmulPerfMode.DoubleRow

    nc = bacc.Bacc("TRN2", target_bir_lowering=False, debug=False, num_devices=NCORES)

    xT_d = nc.dram_tensor("xT", [P, DC, S], fp8, kind="ExternalInput")
    xqT_d = nc.dram_tensor("xqT", [P, DC, SQ], fp8, kind="ExternalInput")
    xres_d = nc.dram_tensor("xres", [P, DC, SQ], f32, kind="ExternalInput")
    wq_d = nc.dram_tensor("wq", [P, DC, D], fp8, kind="ExternalInput")
    wk_d = nc.dram_tensor("wk", [P, DC, D], fp8, kind="ExternalInput")
    wv_d = nc.dram_tensor("wv", [P, DC, D], fp8, kind="ExternalInput")
    wo_d = nc.dram_tensor("wo", [P, DC, D], fp8, kind="ExternalInput")
    w1_d = nc.dram_tensor("w1", [FC, P, DC, P], bf16, kind="ExternalInput")
    w2_d = nc.dram_tensor("w2", [DC, P, FC, P], fp8, kind="ExternalInput")
    bq_d = nc.dram_tensor("bq", [P, DC], f32, kind="ExternalInput")
    bk_d = nc.dram_tensor("bk", [P, DC], f32, kind="ExternalInput")
    bvb_d = nc.dram_tensor("bvb", [P, D], bf16, kind="ExternalInput")
    b1_d = nc.dram_tensor("b1", [P, FC], f32, kind="ExternalInput")
    b2_d = nc.dram_tensor("b2", [P, DC], f32, kind="ExternalInput")
    outT_d = nc.dram_tensor("outT", [P, DC, SQ], f32, kind="ExternalOutput")

    with tile.TileContext(nc) as tc:
        with (
            tc.tile_pool(name="biasp", bufs=1) as biasp,
        ):
            bq_sb = biasp.tile([P, DC], f32)
            bk_sb = biasp.tile([P, DC], f32)
            b1_sb = biasp.tile([P, FC], f32)
            b2_sb = biasp.tile([P, DC], f32)
            nc.sync.dma_start(bq_sb[:], bq_d[:])
            nc.sync.dma_start(bk_sb[:], bk_d[:])
            nc.sync.dma_start(b1_sb[:], b1_d[:])
            nc.sync.dma_start(b2_sb[:], b2_d[:])

            with (
                tc.tile_pool(name="ctxp", bufs=1) as ctxp,
                tc.tile_pool(name="wop", bufs=1) as wop,
            ):
                # ctx carries the SV prescale (from Wv); O-proj descales
                ctxT_sb = ctxp.tile([P, DC, SQ], fp8)
                # bf16(y): input activation of the FFN
                acc_sb = ctxp.tile([P, DC, SQ], bf16)
                wo_sb = wop.tile([P, DC, D], fp8)
                yT_sb = ctxp.tile([P, DC, SQ], f32)

                # ---------------- Phase A+B: QKV projections + attention ----
                with (
                    tc.tile_pool(name="psI", bufs=1, space="PSUM") as psI,
                    tc.tile_pool(name="xrp", bufs=3) as xrp,
                    tc.tile_pool(name="abp", bufs=1) as abp,
                    tc.tile_pool(name="wvp", bufs=1) as wvp,
                    tc.tile_pool(name="ws", bufs=4) as ws,
                    tc.tile_pool(name="ep", bufs=3) as ep,
                    tc.tile_pool(name="ktp", bufs=2) as ktp,
                    tc.tile_pool(name="qtp", bufs=2) as qtp,
                    tc.tile_pool(name="rp", bufs=2) as rp,
                    tc.tile_pool(name="rbp", bufs=2) as rbp,
                ):
                    xTs = [
                        abp.tile([P, DC, 512], fp8, tag=f"xT{c}", name=f"xT{c}")
                        for c in range(4)
                    ]
                    wvs = [
                        wvp.tile([P, DC, 512], fp8, tag=f"wv{c}", name=f"wv{c}")
                        for c in range(2)
                    ]
                    bvb_sb = abp.tile([P, D], bf16)
                    xqT_sb = abp.tile([P, DC, SQ], fp8)
                    # startup DMA order: first the tiles the prologue compute
                    # needs (xTs[0], wv0, wk0/wq0, xqT), then the rest.
                    for k in range(DC):
                        nc.sync.dma_start(
                            xTs[0][:, k : k + 1, :], xT_d[:, k : k + 1, 0:512]
                        )
                        nc.gpsimd.dma_start(
                            wvs[0][:, k : k + 1, :], wv_d[:, k : k + 1, 0:512]
                        )
                    wkts = {0: ws.tile([P, DC, P], fp8, tag="wchunk", name="wkt0")}
                    nc.sync.dma_start(wkts[0][:], wk_d[:, :, 0:P])
                    wqts = {0: ws.tile([P, DC, P], fp8, tag="wchunk", name="wqt0")}
                    nc.sync.dma_start(wqts[0][:], wq_d[:, :, 0:P])
                    nc.sync.dma_start(xqT_sb[:, :, 0:512], xqT_d[:, :, 0:512])
                    nc.sync.dma_start(bvb_sb[:], bvb_d[:])
                    for c in range(1, 4):
                        nc.sync.dma_start(xTs[c][:], xT_d[:, :, c * 512 : (c + 1) * 512])
                    nc.sync.dma_start(xqT_sb[:, :, 512:1024], xqT_d[:, :, 512:1024])
                    nc.sync.dma_start(wvs[1][:], wv_d[:, :, 512:1024])
                    nc.sync.dma_start(wo_sb[:], wo_d[:])

                    # V, token-major: v[sk, dv] (+ ones column per head for
                    # the softmax denominator). Carries the SV prescale.
                    v_sb = abp.tile([P, SKC, H, DH + 1], fp8)
                    nc.vector.memset(v_sb[:, :, :, DH : DH + 1], 1.0)

                    # PE warmup: dummy matmuls ride the startup-DMA shadow so
                    # the tensor engine is at full DVFS clock when the real
                    # prologue work arrives.
                    warm = abp.tile([P, 2, 512], bf16)
                    nc.vector.memset(warm[:, 0:1, 0:4], 0.0)
                    with tc.tile_pool(name="psW", bufs=2, space="PSUM") as psW:
                        for _ in range(10):
                            pw = psW.tile([P, 512], f32, tag="pw", name="pw")
                            for k in range(4):
                                nc.tensor.matmul(
                                    pw,
                                    warm[:, 0, 0:128],
                                    warm[:, 0, :],
                                    start=(k == 0),
                                    stop=(k == 3),
                                )

                    # ---- injection queue: closures emitting one PE matmul
                    # (or a group's DVE tail); drip-fed into attention gaps.
                    pending = []
                    deferred_norm = []
                    done_keys = set()
                    cur_pool = [None]

                    def _run_one():
                        key, fn = pending.pop(0)
                        fn()
                        if key is not None:
                            done_keys.add(key)

                    def pump(k):
                        n = 0
                        while pending and n < k:
                            _run_one()
                            n += 1

                    def require(key):
                        while key not in done_keys and pending:
                            _run_one()

                    def drain():
                        while pending:
                            _run_one()

                    def v_group(nv, sk):
                        xt = xTs[sk // 4]
                        co = (sk % 4) * P
                        box = {}
                        items = []
                        for t in range(DC // 2):
                            def mm(t=t, box=box, xt=xt, co=co, nv=nv):
                                if t == 0:
                                    box["ps"] = cur_pool[0].tile([P, 512], f32, tag="ps", name="ps")
                                nc.tensor.matmul(
                                    box["ps"],
                                    xt[:, 2 * t : 2 * t + 2, co : co + P],
                                    wvs[nv][:, 2 * t : 2 * t + 2, :],
                                    start=(t == 0),
                                    stop=(t == DC // 2 - 1),
                                    perf_mode=DRM,
                                )
                            items.append((None, mm))

                        def tail(box=box, sk=sk, nv=nv):
                            nc.vector.tensor_add(
                                v_sb[:, sk, nv * 8 : nv * 8 + 8, 0:DH],
                                box["ps"].rearrange("p (h d) -> p h d", h=8),
                                bvb_sb[:, nv * 512 : (nv + 1) * 512].rearrange(
                                    "p (h d) -> p h d", h=8
                                ),
                            )
                        items.append((("v", nv, sk), tail))
                        return items

                    def kq_groups(p, kt, qt, wkt, wqt):
                        items = []
                        for n in range(S // 512):
                            box = {}
                            for t in range(DC // 2):
                                def mm(n=n, t=t, box=box, wkt=wkt):
                                    if t == 0:
                                        box["ps"] = cur_pool[0].tile([P, 512], f32, tag="ps", name="ps")
                                    nc.tensor.matmul(
                                        box["ps"],
                                        wkt[:, 2 * t : 2 * t + 2, :],
                                        xTs[n][:, 2 * t : 2 * t + 2, :],
                                        start=(t == 0),
                                        stop=(t == DC // 2 - 1),
                                        perf_mode=DRM,
                                    )
                                items.append((None, mm))

                            def tail(n=n, box=box, kt=kt, p=p):
                                nc.vector.tensor_scalar(
                                    kt[:, n * 512 : (n + 1) * 512],
                                    box["ps"],
                                    1.0 / SW,
                                    bk_sb[:, p : p + 1],
                                    ALU.mult,
                                    ALU.add,
                                )
                            items.append((("kn", p, n), tail))
                        for n in range(SQ // 512):
                            box = {}
                            for t in range(DC // 2):
                                def mm(n=n, t=t, box=box, wqt=wqt):
                                    if t == 0:
                                        box["ps"] = cur_pool[0].tile([P, 512], f32, tag="ps", name="ps")
                                    nc.tensor.matmul(
                                        box["ps"],
                                        wqt[:, 2 * t : 2 * t + 2, :],
                                        xqT_sb[:, 2 * t : 2 * t + 2, n * 512 : (n + 1) * 512],
                                        start=(t == 0),
                                        stop=(t == DC // 2 - 1),
                                        perf_mode=DRM,
                                    )
                                items.append((None, mm))

                            def tail(n=n, box=box, qt=qt, p=p):
                                nc.vector.tensor_scalar(
                                    qt[:, n * 512 : (n + 1) * 512],
                                    box["ps"],
                                    1.0 / SW,
                                    bq_sb[:, p : p + 1],
                                    ALU.mult,
                                    ALU.add,
                                )
                            items.append((("qn", p, n), tail))
                            if n == SQ // 512 - 1:
                                items.append((("kq", p), lambda: None))
                        return items

                    def oa_group(m, n):
                        # partial O-projection: ctx chunks 0-5 (final once
                        # pair 5 is done); writes yT = xres + psum/(SW*SV).
                        # Phase C adds the remaining chunk pair (6,7).
                        box = {}
                        items = []
                        for t in range(3):
                            def mm(t=t, box=box, m=m, n=n):
                                if t == 0:
                                    xr = xrp.tile([P, 512], f32, tag="xr", name="xr")
                                    box["xr"] = xr
                                    nc.sync.dma_start(
                                        xr[:], xres_d[:, m, n * 512 : (n + 1) * 512]
                                    )
                                    box["ps"] = cur_pool[0].tile(
                                        [P, 512], f32, tag="ps", name="ps"
                                    )
                                nc.tensor.matmul(
                                    box["ps"],
                                    wo_sb[:, 2 * t : 2 * t + 2, m * P : (m + 1) * P],
                                    ctxT_sb[:, 2 * t : 2 * t + 2, n * 512 : (n + 1) * 512],
                                    start=(t == 0),
                                    stop=(t == 2),
                                    perf_mode=DRM,
                                )
                            items.append((None, mm))

                        def tail(box=box, m=m, n=n):
                            nc.vector.affine_then_add(
                                yT_sb[:, m, n * 512 : (n + 1) * 512],
                                box["ps"],
                                box["xr"],
                                1.0 / (SW * SV),
                                0.0,
                            )
                        items.append((("oa", m, n), tail))
                        return items

                    # V(nv=1) is needed from pair 4 on; queue it during pairs
                    # 1-4 (behind that pair's K/Q so kt/qt stay just-in-time).
                    V1_QUEUE = {1: range(0, 5), 2: range(5, 10), 3: range(10, 14), 4: range(14, 16)}

                    kts = {0: ktp.tile([P, S], bf16, tag="kt", name="kt")}
                    qts = {0: qtp.tile([P, SQ], bf16, tag="qt", name="qt")}

                    # Prologue: only the first K/Q chunk of pair 0 runs
                    # synchronously so exps can start ASAP; everything else
                    # (K n1-3, Q n1, all of V0) goes through the queue with
                    # fine-grained require() keys guarding each consumer.
                    cur_pool[0] = psI
                    kq0 = kq_groups(0, kts[0], qts[0], wkts[0], wqts[0])
                    for key, it in kq0[:5] + kq0[20:25]:  # K n0, Q n0
                        it()
                        if key is not None:
                            done_keys.add(key)
                    rest = kq0[5:20] + kq0[25:]
                    v0 = [v_group(0, sk) for sk in range(SKC)]
                    pending.extend(rest[0:5])        # K n1
                    pending.extend(v0[0] + v0[1])
                    pending.extend(rest[5:10])       # K n2
                    pending.extend(v0[2] + v0[3])
                    pending.extend(rest[10:15])      # K n3
                    pending.extend(v0[4] + v0[5])
                    pending.extend(rest[15:])        # Q n1 + kq key
                    for sk in range(6, SKC):
                        pending.extend(v0[sk])

                    ps_loop = tc.tile_pool(name="psS", bufs=2, space="PSUM")
                    pc_loop = tc.tile_pool(name="psC", bufs=3, space="PSUM")
                    psS = ps_loop.__enter__()
                    psC = pc_loop.__enter__()
                    for p in range(NPAIR):
                        if p + 1 < NPAIR:
                            kts[p + 1] = ktp.tile([P, S], bf16, tag="kt", name="kt")
                            qts[p + 1] = qtp.tile([P, SQ], bf16, tag="qt", name="qt")
                            wkts[p + 1] = ws.tile([P, DC, P], fp8, tag="wchunk", name="wkt")
                            nc.sync.dma_start(
                                wkts[p + 1][:], wk_d[:, :, (p + 1) * P : (p + 2) * P]
                            )
                            wqts[p + 1] = ws.tile([P, DC, P], fp8, tag="wchunk", name="wqt")
                            nc.sync.dma_start(
                                wqts[p + 1][:], wq_d[:, :, (p + 1) * P : (p + 2) * P]
                            )
                            pending.extend(
                                kq_groups(
                                    p + 1, kts[p + 1], qts[p + 1],
                                    wkts[p + 1], wqts[p + 1],
                                )
                            )
                        if p in V1_QUEUE:
                            for sk in V1_QUEUE[p]:
                                pending.extend(v_group(1, sk))
                        if p == NPAIR - 1:
                            for n in range(SQ // 512):
                                for m in range(DC):
                                    pending.extend(oa_group(m, n))
                        require(("kq", p))
                        kt = kts.pop(p)
                        qt = qts.pop(p)

                        # Attention for head pair (2p, 2p+1). Key chunks are
                        # processed two at a time (parity pairs) so the ctx
                        # contraction can run DoubleRow over 256 keys/instr.
                        for sqn in range(SQ // 512):
                            pc0 = psC.tile([P, 512], f32, tag="pc")
                            pc1 = psC.tile([P, 512], f32, tag="pc")
                            Es = {}

                            def emit_ctx(tt, Es, pc0, pc1, p):
                                require(("v", p // 4, 2 * tt + 1))
                                E = Es.pop(tt)
                                nc.tensor.matmul(
                                    pc0[:65],
                                    v_sb[:, 2 * tt : 2 * tt + 2, 2 * p, :],
                                    E[:, :, 0, :],
                                    start=(tt == 0),
                                    stop=(tt == SKC // 2 - 1),
                                    perf_mode=DRM,
                                )
                                nc.tensor.matmul(
                                    pc1[:65],
                                    v_sb[:, 2 * tt : 2 * tt + 2, 2 * p + 1, :],
                                    E[:, :, 1, :],
                                    start=(tt == 0),
                                    stop=(tt == SKC // 2 - 1),
                                    perf_mode=DRM,
                                )

                            for t in range(SKC // 2):
                                # E[key, parity, head, q] fp8 for the chunk pair
                                E = ep.tile([P, 2, 2, 512], fp8, tag="E")
                                Es[t] = E
                                sss = []
                                if p == 0:
                                    require(("kn", 0, (2 * t + 1) // 4))
                                    require(("qn", 0, sqn))
                                for par in range(2):
                                    sk = 2 * t + par
                                    ss = psS.tile([P, 2, 512], f32)
                                    sss.append(ss)
                                    nc.tensor.matmul(
                                        ss[:, 0, :],
                                        kt[0:64, sk * P : (sk + 1) * P],
                                        qt[0:64, sqn * 512 : (sqn + 1) * 512],
                                        start=True,
                                        stop=True,
                                    )
                                    nc.tensor.matmul(
                                        ss[:, 1, :],
                                        kt[64:128, sk * P : (sk + 1) * P],
                                        qt[64:128, sqn * 512 : (sqn + 1) * 512],
                                        start=True,
                                        stop=True,
                                    )
                                    if par == 0:
                                        pump(1)
                                # both exps back-to-back on ACT; ctx for the
                                # PREVIOUS chunk pair (its E is already done)
                                # keeps PE free of unsatisfied waits.
                                nc.scalar.activation(E[:, 0], sss[0], AF.Exp)
                                nc.scalar.activation(E[:, 1], sss[1], AF.Exp)
                                if t == 0:
                                    # previous sqn's tail: last ctx pair +
                                    # normalize, deferred past this sqn's
                                    # first scores/exps so ACT never stalls
                                    # at the boundary.
                                    for fn in deferred_norm:
                                        fn()
                                    deferred_norm.clear()
                                pump(1)
                                if t > 0:
                                    emit_ctx(t - 1, Es, pc0, pc1, p)
                                pump(1)

                            def normalize(p=p, sqn=sqn, pc0=pc0, pc1=pc1):
                                # softmax normalization: ctx / rowsum (approx
                                # recip is ~18 correct bits, plenty here)
                                s0 = rp.tile([1, 512], f32, tag="s", name="s0")
                                nc.vector.tensor_copy(s0, pc0[64:65, :])
                                r0 = rp.tile([1, 512], f32, tag="r", name="r0")
                                nc.vector.reciprocal_approx_fast(r0, s0)
                                rb0 = rbp.tile([64, 512], f32, tag="rb", name="rb0")
                                nc.gpsimd.partition_broadcast(rb0, r0)
                                nc.vector.tensor_mul(
                                    ctxT_sb[0:64, p, sqn * 512 : (sqn + 1) * 512],
                                    pc0[0:64, :],
                                    rb0,
                                )
                                s1 = rp.tile([1, 512], f32, tag="s", name="s1")
                                nc.vector.tensor_copy(s1, pc1[64:65, :])
                                r1 = rp.tile([1, 512], f32, tag="r", name="r1")
                                nc.vector.reciprocal_approx_fast(r1, s1)
                                rb1 = rbp.tile([64, 512], f32, tag="rb", name="rb1")
                                nc.gpsimd.partition_broadcast(rb1, r1)
                                nc.vector.tensor_mul(
                                    ctxT_sb[64:128, p, sqn * 512 : (sqn + 1) * 512],
                                    pc1[0:64, :],
                                    rb1,
                                )

                            # defer the last ctx pair + normalize into the
                            # next sqn's first slot so ACT rolls straight into
                            # the next exps at the boundary (pc ring 2 stays
                            # legal: the next sqn's first ctx emits at t=1).
                            def tail_fn(Es=Es, pc0=pc0, pc1=pc1, p=p, normalize=normalize):
                                emit_ctx(SKC // 2 - 1, Es, pc0, pc1, p)
                                normalize()

                            deferred_norm.append(tail_fn)
                    drain()
                    for fn in deferred_norm:
                        fn()
                    deferred_norm.clear()
                    pc_loop.__exit__(None, None, None)
                    ps_loop.__exit__(None, None, None)

                # ---------------- Phase C: O projection + residual ----------
                # n-outer so the first half of acc completes early and FFN1
                # can start while the second half is still being produced.
                with (
                    tc.tile_pool(name="psD", bufs=3, space="PSUM") as psD,
                ):
                    for n in range(SQ // 512):
                        for m in range(DC):
                            ps = psD.tile([P, 512], f32)
                            t = DC // 2 - 1
                            nc.tensor.matmul(
                                ps,
                                wo_sb[:, 2 * t : 2 * t + 2, m * P : (m + 1) * P],
                                ctxT_sb[:, 2 * t : 2 * t + 2, n * 512 : (n + 1) * 512],
                                start=True,
                                stop=True,
                                perf_mode=DRM,
                            )
                            # add the last chunk pair onto the partial y
                            ysl = yT_sb[:, m, n * 512 : (n + 1) * 512]
                            nc.vector.affine_then_add(
                                ysl, ps, ysl, 1.0 / (SW * SV), 0.0
                            )
                            # bf16(y) into acc_sb: the FFN input activation
                            nc.scalar.activation(
                                acc_sb[:, m, n * 512 : (n + 1) * 512],
                                ysl,
                                AF.Copy,
                            )

                    # ---------------- Phase D: FFN layer 1 + gelu -----------
                    with tc.tile_pool(name="htp", bufs=1) as htp:
                        hT_sb = htp.tile([P, FC, SQ], fp8)
                        with (
                            tc.tile_pool(name="w2s", bufs=2) as w2s,
                            tc.tile_pool(name="outp", bufs=4) as outp,
                            tc.tile_pool(name="w1s", bufs=3) as w1s,
                        ):
                            w2t0 = w2s.tile([P, FC, P], fp8, tag="w2c", name="w2t0")
                            nc.sync.dma_start(w2t0[:], w2_d[0])
                            for m in range(FC):
                                w1t = w1s.tile([P, DC, P], bf16, tag="w1c")
                                nc.sync.dma_start(w1t[:], w1_d[m])
                                for n in range(SQ // 512):
                                    ps = psD.tile([P, 512], f32)
                                    for k in range(DC):
                                        nc.tensor.matmul(
                                            ps,
                                            w1t[:, k, :],
                                            acc_sb[:, k, n * 512 : (n + 1) * 512],
                                            start=(k == 0),
                                            stop=(k == DC - 1),
                                        )
                                    nc.scalar.activation(
                                        hT_sb[:, m, n * 512 : (n + 1) * 512],
                                        ps,
                                        AF.Gelu,
                                        bias=b1_sb[:, m : m + 1],
                                    )

                            # ------------ Phase E: FFN layer 2 + residual ---
                            for m in range(DC):
                                if m == 0:
                                    w2t = w2t0
                                else:
                                    w2t = w2s.tile([P, FC, P], fp8, tag="w2c", name="w2t")
                                    nc.sync.dma_start(w2t[:], w2_d[m])
                                for n in range(SQ // 512):
                                    ps = psD.tile([P, 512], f32)
                                    for t in range(FC // 2):
                                        nc.tensor.matmul(
                                            ps,
                                            w2t[:, 2 * t : 2 * t + 2, :],
                                            hT_sb[:, 2 * t : 2 * t + 2, n * 512 : (n + 1) * 512],
                                            start=(t == 0),
                                            stop=(t == FC // 2 - 1),
                                            perf_mode=DRM,
                                        )
                                    ot = outp.tile([P, 512], f32, tag="ot")
                                    # psum carries SW; descale + bias in one op
                                    nc.vector.tensor_scalar(
                                        ot, ps, 1.0 / SW, b2_sb[:, m : m + 1],
                                        ALU.mult, ALU.add,
                                    )
                                    nc.vector.tensor_add(
                                        ot, ot, yT_sb[:, m, n * 512 : (n + 1) * 512]
                                    )
                                    nc.sync.dma_start(
                                        outT_d[:, m, n * 512 : (n + 1) * 512], ot
                                    )

    nc.compile()
    return nc


def _get_program():
    if "nc" not in _CACHE:
        _CACHE["nc"] = _build_program()
    return _CACHE["nc"]


def _wlayout(W):
    # [D_in, D_out] -> [P, D_in//P, D_out]
    return np.ascontiguousarray(
        W.reshape(W.shape[0] // P, P, W.shape[1]).transpose(1, 0, 2)
    )


def _blayout(b):
    # [D] -> [P, D//P]
    return np.ascontiguousarray(b.reshape(b.shape[0] // P, P).T)


def prepare_in_maps(x, Wq, bq, Wk, bk, Wv, bv, Wo, bo, W1, b1, W2, b2):
    x = np.asarray(x, np.float32)
    Wq = np.asarray(Wq, np.float32)
    bq = np.asarray(bq, np.float32)
    Wk = np.asarray(Wk, np.float32)
    bk = np.asarray(bk, np.float32)
    Wv = np.asarray(Wv, np.float32)
    bv = np.asarray(bv, np.float32)
    Wo = np.asarray(Wo, np.float32)
    bo = np.asarray(bo, np.float32)
    W1 = np.asarray(W1, np.float32)
    b1 = np.asarray(b1, np.float32)
    W2 = np.asarray(W2, np.float32)
    b2 = np.asarray(b2, np.float32)

    scale = DH ** -0.5
    shared = {
        "wq": _wlayout(Wq * (scale * SW)).astype(F8),
        "wk": _wlayout(Wk * SW).astype(F8),
        "wv": _wlayout(Wv * SV).astype(F8),
        "wo": _wlayout(Wo * SW).astype(F8),
        "w1": np.ascontiguousarray(
            W1.reshape(DC, P, FC, P).transpose(2, 1, 0, 3)
        ).astype(BF16),
        "w2": np.ascontiguousarray(
            (W2 * SW).reshape(FC, P, DC, P).transpose(2, 1, 0, 3)
        ).astype(F8),
        "bq": _blayout(bq * scale),
        "bk": _blayout(bk),
        "bvb": np.ascontiguousarray(np.broadcast_to(bv * SV, (P, D))).astype(BF16),
        "b1": _blayout(b1),
        "b2": _blayout(b2),
    }

    in_maps = []
    for c in range(NCORES):
        b_idx, half = divmod(c, 2)
        xb = x[b_idx]  # [S, D]
        xbT = xb.T  # [D, S]
        xT = np.ascontiguousarray(
            xbT.reshape(DC, P, S).transpose(1, 0, 2)
        ).astype(F8)
        xqT = np.ascontiguousarray(
            xbT[:, half * SQ : (half + 1) * SQ]
            .reshape(DC, P, SQ)
            .transpose(1, 0, 2)
        ).astype(F8)
        xres = np.ascontiguousarray(
            (xbT[:, half * SQ : (half + 1) * SQ] + bo[:, None])
            .reshape(DC, P, SQ)
            .transpose(1, 0, 2)
        ).astype(np.float32)
        in_maps.append(dict(shared, xT=xT, xqT=xqT, xres=xres))
    return in_maps


def assemble_out(results):
    out = np.empty((B, S, D), np.float32)
    for c in range(NCORES):
        b_idx, half = divmod(c, 2)
        outT = results[c]["outT"]  # [P, DC, SQ]
        out[b_idx, half * SQ : (half + 1) * SQ] = (
            outT.transpose(1, 0, 2).reshape(D, SQ).T
        )
    return out


def kernel(**inputs):
    from concourse.bass_utils import run_bass_kernel_spmd

    in_maps = prepare_in_maps(**inputs)
    nc = _get_program()
    res = run_bass_kernel_spmd(nc, in_maps, core_ids=list(range(NCORES)))
    return assemble_out(res.results)
